# revision 44
# baseline (speedup 1.0000x reference)
"""DenseGCN (3x EdgeConv max-aggr) on 8 TRN2 NeuronCores.

Key algebra: EdgeConv message relu([x_d, x_s - x_d] @ W + b) with W=[Wt;Wb]
= relu(x_d@(Wt-Wb) + x_s@Wb + b). relu and the per-dst add are monotone, so
segment_max commutes: agg[v] = relu(A[v] + b + max_{e:dst=v} B[src_e]) with
A = x@(Wt-Wb) + b, B = x@Wb -- node-level matmuls only, no per-edge matmul.

Sharding: dst-partition nodes (2500/core, padded to 2560 slots, sorted by
in-degree desc). Per block: one merged [C,128] matmul per node tile yields
[A|B] in PSUM, AllGather of the bf16 B table, then a handful of giant
multi-column indirect-DMA gathers into a bf16 SBUF slab (amortizes the ~1us
fixed SWDGE cost that dominated the per-round version), DVE tree-folds for
the segment max, and a final quad max-pool.
"""
import sys, os, types

sys.path.insert(0, "/opt/trn_rl_repo")

import numpy as np


def _setup_trace_hook():
    """Register the NTFF profile hook (missing antenv.axon_hooks shim) so
    run_bass_kernel_spmd(trace=True) works. Safe no-op on failure."""
    try:
        import antenv

        if "antenv.axon_hooks" not in sys.modules:
            m = types.ModuleType("antenv.axon_hooks")
            hook = [None]
            m.set_axon_ntff_profile_hook = lambda h: hook.__setitem__(0, h)
            m.get_axon_ntff_profile_hook = lambda: hook[0]
            sys.modules["antenv.axon_hooks"] = m
            antenv.axon_hooks = m
            from trn_agent_boot.trn_boot import _ntff_profile_via_ctypes

            m.set_axon_ntff_profile_hook(
                _ntff_profile_via_ctypes("/opt/axon/libaxon_pjrt.so")
            )
        import concourse.bass_utils as bu

        bu.upload_artifacts = lambda tmpdir: tmpdir
        return True
    except Exception:
        return False

N_NODES = 20000
N_EDGES = 320000
C_IN = 64
GROWTH = 64
N_BLOCKS = 3
N_CORES = 8
NL = N_NODES // N_CORES          # 2500 local nodes
SLOTS = 2560                     # padded to 20*128
JBLK = SLOTS // 128              # 20 slot-blocks
TBL_STRIDE = SLOTS               # per-core chunk in the AllGathered table
TBL_ROWS = N_CORES * TBL_STRIDE + 64  # + slack: the AG tail clobbers past-end
NGRP = 4                         # gather calls per block (pipeline granularity)


def _build_grid(edge_index):
    """Host-side index manipulation: degree-sort nodes per core, build the
    per-slot-block column layout (R_c columns for block c) and the int32
    gather index grid [8, 128, K], identical structure across cores."""
    src, dst = edge_index[0].astype(np.int64), edge_index[1].astype(np.int64)
    core_of = dst // NL
    perms = []
    slot_of = np.full(N_NODES, -1, np.int64)
    for m in range(N_CORES):
        lo, hi = m * NL, (m + 1) * NL
        deg = np.bincount(dst[(dst >= lo) & (dst < hi)] - lo, minlength=NL)
        order = np.argsort(-deg, kind="stable")
        nodes = order + lo
        perm = np.concatenate([nodes, np.full(SLOTS - NL, -1, np.int64)])
        perms.append(perm)
        slot_of[nodes] = np.arange(NL)
    grow = (np.arange(N_NODES) // NL) * TBL_STRIDE + slot_of  # node -> table row

    per_core = []
    degs = np.zeros((N_CORES, SLOTS), np.int64)
    for m in range(N_CORES):
        lo = m * NL
        mask = core_of == m
        s_rows = grow[src[mask]]
        sl = slot_of[dst[mask]]
        o = np.argsort(sl, kind="stable")
        sl, s_rows = sl[o], s_rows[o]
        starts = np.searchsorted(sl, np.arange(SLOTS))
        ends = np.searchsorted(sl, np.arange(SLOTS) + 1)
        degs[m] = ends - starts
        per_core.append((sl, s_rows, starts))

    # columns per slot-block: R_c = max degree within block across cores (>=1)
    Rs = [max(1, int(degs[:, c * 128 : (c + 1) * 128].max())) for c in range(JBLK)]
    col_base = np.concatenate([[0], np.cumsum(Rs)]).astype(np.int64)
    K = int(col_base[-1])

    # Pad a slot's unused rounds by REPEATING its first source row (max is
    # idempotent) -- repeated gathers of one shared dummy row misbehave in the
    # HW SWDGE ucode. Zero-degree slots (only the discarded pad slots here)
    # point at row 0; their outputs never leave the device unshuffled range.
    idx_all = np.full((N_CORES, 128, K), -1, np.int32)
    for m in range(N_CORES):
        sl, s_rows, starts = per_core[m]
        r = np.arange(len(sl)) - starts[sl]          # round of each edge
        cols = col_base[sl // 128] + r
        idx_all[m, sl % 128, cols] = s_rows
        first_src = np.zeros(SLOTS, np.int64)
        has = degs[m] > 0
        first_src[has] = s_rows[starts[np.where(has)[0]]]
        fs_pj = np.zeros((128, K), np.int64)
        for c in range(JBLK):
            fs_pj[:, col_base[c] : col_base[c + 1]] = first_src[
                c * 128 : (c + 1) * 128
            ][:, None]
        grid = idx_all[m]
        idx_all[m] = np.where(grid < 0, fs_pj, grid).astype(np.int32)

    # dma_gather int16 index layout, per <=1024-idx call (HW per-call limit):
    # call j covers slab cols [8j, 8j+8); its flat idx i = q*128 + p lives at
    # idx16[i % 16, 64j + i // 16], replicated across the 8 Q7 stripes.
    K_pad = ((K + 7) // 8) * 8
    ncalls = K_pad // 8
    idx16 = np.zeros((N_CORES, 128, K_pad * 8), np.int16)
    for m in range(N_CORES):
        flat = np.zeros(K_pad * 128, np.int64)
        flat[: K * 128] = idx_all[m].T.reshape(-1)    # (q*128 + p) order
        for j in range(ncalls):
            seg = flat[j * 1024 : (j + 1) * 1024]
            wrap = seg.reshape(-1, 16).T.astype(np.int16)   # [16, 64]
            idx16[m, :, j * 64 : (j + 1) * 64] = np.tile(wrap, (8, 1))
    return perms, Rs, col_base, K, K_pad, idx_all, idx16


def kernel(x, W0, b0, W1, b1, W2, b2, edge_index):
    import concourse.bacc as bacc
    import concourse.bass as bass
    import concourse.mybir as mybir
    import concourse.tile as tile
    from concourse.tile import add_dep_helper
    from concourse.masks import make_identity
    from concourse.bass_utils import run_bass_kernel_spmd
    from concourse.library_config import mlp as mlp_lib

    x = np.asarray(x, np.float32)
    Ws = [np.asarray(W0, np.float32), np.asarray(W1, np.float32), np.asarray(W2, np.float32)]
    bs = [np.asarray(b0, np.float32), np.asarray(b1, np.float32), np.asarray(b2, np.float32)]
    edge_index = np.asarray(edge_index)

    perms, Rs, col_base, K, K_pad, idx_all, idx16 = _build_grid(edge_index)
    NCALLS = K_pad // 8
    CS = [C_IN + k * GROWTH for k in range(N_BLOCKS)]   # 64,128,192

    # runs of adjacent slot-blocks with equal R (for batched tree folds)
    runs = []  # (c0, ncs, R)
    c = 0
    while c < JBLK:
        c2 = c
        while c2 + 1 < JBLK and Rs[c2 + 1] == Rs[c]:
            c2 += 1
        runs.append((c, c2 - c + 1, Rs[c]))
        c = c2 + 1

    f32 = mybir.dt.float32
    nc = bacc.Bacc(
        "TRN2", target_bir_lowering=False, debug=False, num_devices=N_CORES,
        dynamic_dma_scratch_size=32768, num_swdge_queues=4,
    )

    XT0in = nc.declare_dram_parameter("XT0in", [64, SLOTS], f32, isOutput=False)
    Xnm = nc.declare_dram_parameter("Xnm", [128, JBLK * 64], f32, isOutput=False)
    # Wc_k: rows 0:C = [Wt-Wb | Wb] ([C,128]), row C = [b | 0]
    Wp = [nc.declare_dram_parameter(f"W{k}", [CS[k] + 1, 128], f32, isOutput=False) for k in range(3)]
    IdxP = nc.declare_dram_parameter("IdxP", [128, K_pad * 8], mybir.dt.int16, isOutput=False)
    OutP = nc.declare_dram_parameter("Out", [SLOTS, 64], f32, isOutput=True)

    Btabs = [nc.dram_tensor(f"btab{i}", [TBL_ROWS, 64], f32) for i in range(2)]
    Bloc = nc.dram_tensor("bloc", [TBL_STRIDE, 64], f32)

    KDBG = os.environ.get("KDBG", "0") == "1"
    NBRUN = int(os.environ.get("KBLOCKS", "3"))
    if KDBG:
        DbgSlab = nc.declare_dram_parameter("DbgSlab", [128, K * 64], f32, isOutput=True)
        DbgAs = nc.declare_dram_parameter("DbgAs", [128, JBLK * 64], f32, isOutput=True)
        DbgXc = nc.declare_dram_parameter("DbgXc", [128, JBLK * 256], f32, isOutput=True)

    # W chunk map per block: (xt_tile_idx, xt_row0, kk, wslot)
    chunk_map = {
        0: [(0, 0, 64, 0)],
        1: [(0, 0, 128, 1)],
        2: [(0, 0, 128, 2), (1, 0, 64, 3)],
    }
    NWSLOT = 4

    with tile.TileContext(nc) as tc:
        with (
            tc.tile_pool(name="big", bufs=1) as bigp,
            tc.tile_pool(name="ps", bufs=4, space="PSUM") as psp,
        ):
            xT0 = bigp.tile([128, SLOTS], f32, tag="xT0")
            xT1 = bigp.tile([128, SLOTS], f32, tag="xT1")
            Xcat = bigp.tile([128, JBLK, 256], f32, tag="Xcat")
            As = bigp.tile([128, JBLK, 64], f32, tag="As")
            Bs = bigp.tile([128, JBLK, 64], f32, tag="Bs")
            slab = bigp.tile([128, K_pad, 64], f32, tag="slab")
            Idx = bigp.tile([128, K_pad * 8], mybir.dt.int16, tag="Idx")
            Wt = bigp.tile([128, NWSLOT * 128], f32, tag="Wt")
            ident = bigp.tile([128, 128], f32, tag="ident")

            ll = nc.gpsimd.load_library(mlp_lib)  # dma_gather ucode
            make_identity(nc, ident[:])
            nc.sync.dma_start(out=xT0[0:64, :], in_=XT0in[:])
            nc.sync.dma_start(
                out=Xcat[:, :, 0:64],
                in_=Xnm[:].rearrange("p (j c) -> p j c", c=64),
            )
            nc.sync.dma_start(out=Idx[:], in_=IdxP[:])
            for k in range(3):
                C = CS[k]
                for (xti, row0, kk, wslot) in chunk_map[k]:
                    wr0 = 0 if wslot != 3 else 128
                    nc.sync.dma_start(
                        out=Wt[0:kk, wslot * 128 : (wslot + 1) * 128],
                        in_=Wp[k][wr0 : wr0 + kk, :],
                    )
            war_dep = {0: [], 1: []}
            gidx = 0  # global SWDGE gather counter: keeps Tile's DMASW lane
            #           rotation (mod 8) aligned with our queue rotation (mod 4)
            for k in range(NBRUN):
                Btab = Btabs[k % 2]
                chunks = chunk_map[k]

                # 1) merged [A|B] matmuls; B half -> Bloc asap (gates the AG)
                bws = []
                for t in range(JBLK):
                    ps = psp.tile([128, 128], f32, tag="mm")
                    for ci, (xti, row0, kk, wslot) in enumerate(chunks):
                        xt_tile = xT0 if xti == 0 else xT1
                        nc.tensor.matmul(
                            out=ps[:, :],
                            lhsT=xt_tile[row0 : row0 + kk, t * 128 : (t + 1) * 128],
                            rhs=Wt[0:kk, wslot * 128 : (wslot + 1) * 128],
                            start=(ci == 0),
                            stop=(ci == len(chunks) - 1),
                        )
                    cpB = nc.scalar.copy(out=Bs[:, t, :], in_=ps[:, 64:128])
                    bw = nc.sync.dma_start(
                        out=Bloc[t * 128 : (t + 1) * 128, :], in_=Bs[:, t, :]
                    )
                    bws.append(bw)
                    # A columns already hold x@(Wt-Wb); just move PSUM -> SBUF
                    nc.vector.tensor_copy(out=As[:, t, :], in_=ps[:, 0:64])

                ag = nc.gpsimd.collective_compute(
                    "AllGather", mybir.AluOpType.bypass,
                    replica_groups=[list(range(N_CORES))],
                    ins=[Bloc[:]],
                    outs=[Btab[0 : N_CORES * TBL_STRIDE, :]],
                )
                for bw in bws:
                    add_dep_helper(ag.ins, bw.ins, sync=True, reason="AG after writes")
                for d in war_dep[k % 2]:
                    add_dep_helper(ag.ins, d.ins, sync=True, reason="WAR: prior reads done")

                # 2) SWDGE row-gathers (<=1024 idx/call, 4 queues round-robin)
                gcalls = []
                for j in range(NCALLS):
                    gcall = nc.gpsimd.dma_gather(
                        slab[:, j * 8 : (j + 1) * 8, :],
                        Btab[0 : N_CORES * TBL_STRIDE, :],
                        Idx[:, j * 64 : (j + 1) * 64],
                        1024,
                        1024,
                        64,
                        queue_num=gidx % 4,
                    )
                    gidx += 1
                    add_dep_helper(gcall.ins, ag.ins, sync=True, reason="table ready")
                    if k == 0 and j == 0:
                        add_dep_helper(gcall.ins, ll.ins, sync=True, reason="ucode")
                    gcalls.append(gcall)
                war_dep[k % 2] = gcalls
                if KDBG and k == NBRUN - 1:
                    nc.sync.dma_start(
                        out=DbgSlab[:], in_=slab[:, 0:K, :].rearrange("p q f -> p (q f)")
                    )
                    nc.sync.dma_start(
                        out=DbgAs[:], in_=As[:].rearrange("p j c -> p (j c)")
                    )

                # 3) per run: tree-fold the segments, then epilogue+transpose;
                #    tile per-AP deps pipeline these behind the right calls
                for (c0, ncs, R) in runs:
                    q0 = int(col_base[c0])
                    seg = slab[:, q0 : q0 + ncs * R, :].rearrange(
                        "p (c r) f -> p c r f", r=R
                    )
                    L = R
                    while L > 1:
                        h = L // 2
                        nc.vector.tensor_tensor(
                            out=seg[:, :, 0:h, :],
                            in0=seg[:, :, 0:h, :],
                            in1=seg[:, :, L - h : L, :],
                            op=mybir.AluOpType.max,
                        )
                        L -= h
                    for c in range(c0, c0 + ncs):
                        cb = int(col_base[c])
                        dst = Xcat[:, c, 64 * (k + 1) : 64 * (k + 2)]
                        nc.vector.tensor_tensor(
                            out=dst, in0=As[:, c, :], in1=slab[:, cb, :],
                            op=mybir.AluOpType.add,
                        )
                        e3 = nc.vector.tensor_scalar(
                            out=dst, in0=dst,
                            scalar1=0.0, scalar2=None, op0=mybir.AluOpType.max,
                        )
                        if k < 2:
                            pst = psp.tile([128, 128], f32, tag="tps")
                            tp = nc.tensor.transpose(
                                out=pst[0:64, :],
                                in_=dst,
                                identity=ident[:],
                            )
                            add_dep_helper(tp.ins, e3.ins, sync=True, reason="agg ready")
                            dst_tile = xT0 if k == 0 else xT1
                            ro = 64 if k == 0 else 0
                            nc.scalar.copy(
                                out=dst_tile[ro : ro + 64, c * 128 : (c + 1) * 128],
                                in_=pst[0:64, :],
                            )

            if KDBG:
                nchan = 64 * (NBRUN + 1)
                nc.sync.dma_start(
                    out=DbgXc[:, 0 : JBLK * nchan].rearrange(
                        "p (j c) -> p j c", c=nchan
                    ),
                    in_=Xcat[:, :, 0:nchan],
                )
            # 4) final quad max-pool (into As, free now)
            if NBRUN == 3:
                red = nc.vector.tensor_reduce(
                    out=As[:].rearrange("p j c -> p (j c)"),
                    in_=Xcat[:].rearrange("p j (g f) -> p (j g) f", f=4),
                    op=mybir.AluOpType.max,
                    axis=mybir.AxisListType.X,
                )
                ow = nc.sync.dma_start(
                    out=OutP[:].rearrange("(j p) c -> p j c", p=128),
                    in_=As[:],
                )
                add_dep_helper(ow.ins, red.ins, sync=True, reason="out ready")
            else:
                nc.sync.dma_start(
                    out=OutP[:].rearrange("(j p) c -> p j c", p=128),
                    in_=As[:],
                )

    nc.compile()

    # ---- host-side shard + run ----
    in_maps = []
    for m in range(N_CORES):
        perm = perms[m]
        xp = np.zeros((SLOTS, 64), np.float32)
        sel = perm >= 0
        xp[sel] = x[perm[sel]]
        xnm = np.zeros((128, JBLK, 64), np.float32)
        s = np.arange(SLOTS)
        xnm[s % 128, s // 128, :] = xp
        in_map = {
            "XT0in": np.ascontiguousarray(xp.T),
            "Xnm": xnm.reshape(128, -1),
            "IdxP": idx16[m],
        }
        for k in range(3):
            C = CS[k]
            Wt_, Wb_ = Ws[k][:C], Ws[k][C:]
            comb = np.concatenate([Wt_ - Wb_, Wb_], axis=1)  # [C, 128]
            brow = np.concatenate([bs[k], np.zeros(64, np.float32)])[None, :]
            in_map[f"W{k}"] = np.ascontiguousarray(
                np.vstack([comb, brow]).astype(np.float32)
            )
        in_maps.append(in_map)

    if os.environ.get("BASS_SIM", "0") == "1":
        from concourse.bass_interp import MultiCoreSim

        sim = MultiCoreSim(nc, num_cores=N_CORES, num_workers=8)
        for m, core_sim in sim.cores.items():
            for name, val in in_maps[m].items():
                core_sim.tensor(name)[:] = val
        sim.simulate()
        out = np.zeros((N_NODES, 64), np.float32)
        for m in range(N_CORES):
            o = np.asarray(sim.cores[m].tensor("Out"))
            perm = perms[m]
            sel = perm >= 0
            out[perm[sel]] = o[sel]
        kernel._last_res = None
        return out

    trace = os.environ.get("BASS_KERNEL_TRACE", "0") == "1" and _setup_trace_hook()
    res = run_bass_kernel_spmd(
        nc, in_maps, core_ids=list(range(N_CORES)), trace=trace
    )
    out = np.zeros((N_NODES, 64), np.float32)
    for m in range(N_CORES):
        o = res.results[m]["Out"]
        perm = perms[m]
        sel = perm >= 0
        out[perm[sel]] = o[sel]
    kernel._last_res = res
    return out


# revision 45
# speedup vs baseline: 1.0088x; 1.0088x over previous
"""DenseGCN (3x EdgeConv max-aggr) on 8 TRN2 NeuronCores.

Key algebra: EdgeConv message relu([x_d, x_s - x_d] @ W + b) with W=[Wt;Wb]
= relu(x_d@(Wt-Wb) + x_s@Wb + b). relu and the per-dst add are monotone, so
segment_max commutes: agg[v] = relu(A[v] + b + max_{e:dst=v} B[src_e]) with
A = x@(Wt-Wb) + b, B = x@Wb -- node-level matmuls only, no per-edge matmul.

Sharding: dst-partition nodes (2500/core, padded to 2560 slots, sorted by
in-degree desc). Per block: one merged [C,128] matmul per node tile yields
[A|B] in PSUM, AllGather of the bf16 B table, then a handful of giant
multi-column indirect-DMA gathers into a bf16 SBUF slab (amortizes the ~1us
fixed SWDGE cost that dominated the per-round version), DVE tree-folds for
the segment max, and a final quad max-pool.
"""
import sys, os, types

sys.path.insert(0, "/opt/trn_rl_repo")

import numpy as np


def _setup_trace_hook():
    """Register the NTFF profile hook (missing antenv.axon_hooks shim) so
    run_bass_kernel_spmd(trace=True) works. Safe no-op on failure."""
    try:
        import antenv

        if "antenv.axon_hooks" not in sys.modules:
            m = types.ModuleType("antenv.axon_hooks")
            hook = [None]
            m.set_axon_ntff_profile_hook = lambda h: hook.__setitem__(0, h)
            m.get_axon_ntff_profile_hook = lambda: hook[0]
            sys.modules["antenv.axon_hooks"] = m
            antenv.axon_hooks = m
            from trn_agent_boot.trn_boot import _ntff_profile_via_ctypes

            m.set_axon_ntff_profile_hook(
                _ntff_profile_via_ctypes("/opt/axon/libaxon_pjrt.so")
            )
        import concourse.bass_utils as bu

        bu.upload_artifacts = lambda tmpdir: tmpdir
        return True
    except Exception:
        return False

N_NODES = 20000
N_EDGES = 320000
C_IN = 64
GROWTH = 64
N_BLOCKS = 3
N_CORES = 8
NL = N_NODES // N_CORES          # 2500 local nodes
SLOTS = 2560                     # padded to 20*128
JBLK = SLOTS // 128              # 20 slot-blocks
TBL_STRIDE = SLOTS               # per-core chunk in the AllGathered table
TBL_ROWS = N_CORES * TBL_STRIDE + 64  # + slack: the AG tail clobbers past-end
NGRP = 4                         # gather calls per block (pipeline granularity)


def _build_grid(edge_index):
    """Host-side index manipulation: degree-sort nodes per core, build the
    per-slot-block column layout (R_c columns for block c) and the int32
    gather index grid [8, 128, K], identical structure across cores."""
    src, dst = edge_index[0].astype(np.int64), edge_index[1].astype(np.int64)
    core_of = dst // NL
    perms = []
    slot_of = np.full(N_NODES, -1, np.int64)
    for m in range(N_CORES):
        lo, hi = m * NL, (m + 1) * NL
        deg = np.bincount(dst[(dst >= lo) & (dst < hi)] - lo, minlength=NL)
        order = np.argsort(-deg, kind="stable")
        nodes = order + lo
        perm = np.concatenate([nodes, np.full(SLOTS - NL, -1, np.int64)])
        perms.append(perm)
        slot_of[nodes] = np.arange(NL)
    grow = (np.arange(N_NODES) // NL) * TBL_STRIDE + slot_of  # node -> table row

    per_core = []
    degs = np.zeros((N_CORES, SLOTS), np.int64)
    for m in range(N_CORES):
        lo = m * NL
        mask = core_of == m
        s_rows = grow[src[mask]]
        sl = slot_of[dst[mask]]
        o = np.argsort(sl, kind="stable")
        sl, s_rows = sl[o], s_rows[o]
        starts = np.searchsorted(sl, np.arange(SLOTS))
        ends = np.searchsorted(sl, np.arange(SLOTS) + 1)
        degs[m] = ends - starts
        per_core.append((sl, s_rows, starts))

    # columns per slot-block: R_c = max degree within block across cores (>=1)
    Rs = [max(1, int(degs[:, c * 128 : (c + 1) * 128].max())) for c in range(JBLK)]
    col_base = np.concatenate([[0], np.cumsum(Rs)]).astype(np.int64)
    K = int(col_base[-1])

    # Pad a slot's unused rounds by REPEATING its first source row (max is
    # idempotent) -- repeated gathers of one shared dummy row misbehave in the
    # HW SWDGE ucode. Zero-degree slots (only the discarded pad slots here)
    # point at row 0; their outputs never leave the device unshuffled range.
    idx_all = np.full((N_CORES, 128, K), -1, np.int32)
    for m in range(N_CORES):
        sl, s_rows, starts = per_core[m]
        r = np.arange(len(sl)) - starts[sl]          # round of each edge
        cols = col_base[sl // 128] + r
        idx_all[m, sl % 128, cols] = s_rows
        first_src = np.zeros(SLOTS, np.int64)
        has = degs[m] > 0
        first_src[has] = s_rows[starts[np.where(has)[0]]]
        fs_pj = np.zeros((128, K), np.int64)
        for c in range(JBLK):
            fs_pj[:, col_base[c] : col_base[c + 1]] = first_src[
                c * 128 : (c + 1) * 128
            ][:, None]
        grid = idx_all[m]
        idx_all[m] = np.where(grid < 0, fs_pj, grid).astype(np.int32)

    # dma_gather int16 index layout, per <=1024-idx call (HW per-call limit):
    # call j covers slab cols [8j, 8j+8); its flat idx i = q*128 + p lives at
    # idx16[i % 16, 64j + i // 16], replicated across the 8 Q7 stripes.
    K_pad = ((K + 7) // 8) * 8
    ncalls = K_pad // 8
    idx16 = np.zeros((N_CORES, 128, K_pad * 8), np.int16)
    for m in range(N_CORES):
        flat = np.zeros(K_pad * 128, np.int64)
        flat[: K * 128] = idx_all[m].T.reshape(-1)    # (q*128 + p) order
        for j in range(ncalls):
            seg = flat[j * 1024 : (j + 1) * 1024]
            wrap = seg.reshape(-1, 16).T.astype(np.int16)   # [16, 64]
            idx16[m, :, j * 64 : (j + 1) * 64] = np.tile(wrap, (8, 1))
    return perms, Rs, col_base, K, K_pad, idx_all, idx16


def kernel(x, W0, b0, W1, b1, W2, b2, edge_index):
    import concourse.bacc as bacc
    import concourse.bass as bass
    import concourse.mybir as mybir
    import concourse.tile as tile
    from concourse.tile import add_dep_helper
    from concourse.masks import make_identity
    from concourse.bass_utils import run_bass_kernel_spmd
    from concourse.library_config import mlp as mlp_lib

    x = np.asarray(x, np.float32)
    Ws = [np.asarray(W0, np.float32), np.asarray(W1, np.float32), np.asarray(W2, np.float32)]
    bs = [np.asarray(b0, np.float32), np.asarray(b1, np.float32), np.asarray(b2, np.float32)]
    edge_index = np.asarray(edge_index)

    perms, Rs, col_base, K, K_pad, idx_all, idx16 = _build_grid(edge_index)
    NCALLS = K_pad // 8
    CS = [C_IN + k * GROWTH for k in range(N_BLOCKS)]   # 64,128,192

    # runs of adjacent slot-blocks with equal R (for batched tree folds)
    runs = []  # (c0, ncs, R)
    c = 0
    while c < JBLK:
        c2 = c
        while c2 + 1 < JBLK and Rs[c2 + 1] == Rs[c]:
            c2 += 1
        runs.append((c, c2 - c + 1, Rs[c]))
        c = c2 + 1

    f32 = mybir.dt.float32
    nc = bacc.Bacc(
        "TRN2", target_bir_lowering=False, debug=False, num_devices=N_CORES,
        dynamic_dma_scratch_size=32768, num_swdge_queues=4,
    )

    XT0in = nc.declare_dram_parameter("XT0in", [64, SLOTS], f32, isOutput=False)
    Xnm = nc.declare_dram_parameter("Xnm", [128, JBLK * 64], f32, isOutput=False)
    # Wc_k: rows 0:C = [Wt-Wb | Wb] ([C,128]), row C = [b | 0]
    Wp = [nc.declare_dram_parameter(f"W{k}", [CS[k] + 1, 128], f32, isOutput=False) for k in range(3)]
    IdxP = nc.declare_dram_parameter("IdxP", [128, K_pad * 8], mybir.dt.int16, isOutput=False)
    OutP = nc.declare_dram_parameter("Out", [SLOTS, 64], f32, isOutput=True)

    Btabs = [nc.dram_tensor(f"btab{i}", [TBL_ROWS, 64], f32) for i in range(2)]
    Bloc = nc.dram_tensor("bloc", [TBL_STRIDE, 64], f32)

    KDBG = os.environ.get("KDBG", "0") == "1"
    NBRUN = int(os.environ.get("KBLOCKS", "3"))
    if KDBG:
        DbgSlab = nc.declare_dram_parameter("DbgSlab", [128, K * 64], f32, isOutput=True)
        DbgAs = nc.declare_dram_parameter("DbgAs", [128, JBLK * 64], f32, isOutput=True)
        DbgXc = nc.declare_dram_parameter("DbgXc", [128, JBLK * 256], f32, isOutput=True)

    # W chunk map per block: (xt_tile_idx, xt_row0, kk, wslot)
    chunk_map = {
        0: [(0, 0, 64, 0)],
        1: [(0, 0, 128, 1)],
        2: [(0, 0, 128, 2), (1, 0, 64, 3)],
    }
    NWSLOT = 4

    with tile.TileContext(nc) as tc:
        with (
            tc.tile_pool(name="big", bufs=1) as bigp,
            tc.tile_pool(name="ps", bufs=4, space="PSUM") as psp,
        ):
            xT0 = bigp.tile([128, SLOTS], f32, tag="xT0")
            xT1 = bigp.tile([128, SLOTS], f32, tag="xT1")
            Xcat = bigp.tile([128, JBLK, 256], f32, tag="Xcat")
            As = bigp.tile([128, JBLK, 64], f32, tag="As")
            Bs = bigp.tile([128, JBLK, 64], f32, tag="Bs")
            slab = bigp.tile([128, K_pad, 64], f32, tag="slab")
            Idx = bigp.tile([128, K_pad * 8], mybir.dt.int16, tag="Idx")
            Wt = bigp.tile([128, NWSLOT * 128], f32, tag="Wt")
            ident = bigp.tile([128, 128], f32, tag="ident")

            ll = nc.gpsimd.load_library(mlp_lib)  # dma_gather ucode
            make_identity(nc, ident[:])
            nc.sync.dma_start(out=xT0[0:64, :], in_=XT0in[:])
            nc.sync.dma_start(
                out=Xcat[:, :, 0:64],
                in_=Xnm[:].rearrange("p (j c) -> p j c", c=64),
            )
            nc.sync.dma_start(out=Idx[:], in_=IdxP[:])
            for k in range(3):
                C = CS[k]
                for (xti, row0, kk, wslot) in chunk_map[k]:
                    wr0 = 0 if wslot != 3 else 128
                    nc.sync.dma_start(
                        out=Wt[0:kk, wslot * 128 : (wslot + 1) * 128],
                        in_=Wp[k][wr0 : wr0 + kk, :],
                    )
            war_dep = {0: [], 1: []}
            gidx = 0  # global SWDGE gather counter: keeps Tile's DMASW lane
            #           rotation (mod 8) aligned with our queue rotation (mod 4)
            for k in range(NBRUN):
                Btab = Btabs[k % 2]
                chunks = chunk_map[k]

                # 1) merged [A|B] matmuls; B half -> Bloc asap (gates the AG)
                bws = []
                for t in range(JBLK):
                    ps = psp.tile([128, 128], f32, tag="mm")
                    for ci, (xti, row0, kk, wslot) in enumerate(chunks):
                        xt_tile = xT0 if xti == 0 else xT1
                        nc.tensor.matmul(
                            out=ps[:, :],
                            lhsT=xt_tile[row0 : row0 + kk, t * 128 : (t + 1) * 128],
                            rhs=Wt[0:kk, wslot * 128 : (wslot + 1) * 128],
                            start=(ci == 0),
                            stop=(ci == len(chunks) - 1),
                        )
                    cpB = nc.scalar.copy(out=Bs[:, t, :], in_=ps[:, 64:128])
                    bw = nc.sync.dma_start(
                        out=Bloc[t * 128 : (t + 1) * 128, :], in_=Bs[:, t, :]
                    )
                    bws.append(bw)
                    # A columns already hold x@(Wt-Wb); just move PSUM -> SBUF
                    nc.vector.tensor_copy(out=As[:, t, :], in_=ps[:, 0:64])

                ag = nc.gpsimd.collective_compute(
                    "AllGather", mybir.AluOpType.bypass,
                    replica_groups=[list(range(N_CORES))],
                    ins=[Bloc[:]],
                    outs=[Btab[0 : N_CORES * TBL_STRIDE, :]],
                )
                for bw in bws:
                    add_dep_helper(ag.ins, bw.ins, sync=True, reason="AG after writes")
                for d in war_dep[k % 2]:
                    add_dep_helper(ag.ins, d.ins, sync=True, reason="WAR: prior reads done")

                # 2) SWDGE row-gathers (<=1024 idx/call, 4 queues round-robin)
                gcalls = []
                for j in range(NCALLS):
                    gcall = nc.gpsimd.dma_gather(
                        slab[:, j * 8 : (j + 1) * 8, :],
                        Btab[0 : N_CORES * TBL_STRIDE, :],
                        Idx[:, j * 64 : (j + 1) * 64],
                        1024,
                        1024,
                        64,
                        queue_num=gidx % 4,
                    )
                    gidx += 1
                    add_dep_helper(gcall.ins, ag.ins, sync=True, reason="table ready")
                    if k == 0 and j == 0:
                        add_dep_helper(gcall.ins, ll.ins, sync=True, reason="ucode")
                    gcalls.append(gcall)
                war_dep[k % 2] = gcalls
                if KDBG and k == NBRUN - 1:
                    nc.sync.dma_start(
                        out=DbgSlab[:], in_=slab[:, 0:K, :].rearrange("p q f -> p (q f)")
                    )
                    nc.sync.dma_start(
                        out=DbgAs[:], in_=As[:].rearrange("p j c -> p (j c)")
                    )

                # 3) per run: tree-fold the segments, then epilogue+transpose;
                #    tile per-AP deps pipeline these behind the right calls
                for (c0, ncs, R) in runs:
                    q0 = int(col_base[c0])
                    seg = slab[:, q0 : q0 + ncs * R, :].rearrange(
                        "p (c r) f -> p c r f", r=R
                    )
                    L = R
                    while L > 1:
                        h = L // 2
                        nc.vector.tensor_tensor(
                            out=seg[:, :, 0:h, :],
                            in0=seg[:, :, 0:h, :],
                            in1=seg[:, :, L - h : L, :],
                            op=mybir.AluOpType.max,
                        )
                        L -= h
                    for c in range(c0, c0 + ncs):
                        cb = int(col_base[c])
                        dst = Xcat[:, c, 64 * (k + 1) : 64 * (k + 2)]
                        nc.vector.tensor_tensor(
                            out=dst, in0=As[:, c, :], in1=slab[:, cb, :],
                            op=mybir.AluOpType.add,
                        )
                        e3 = nc.scalar.activation(
                            out=dst, in_=dst,
                            func=mybir.ActivationFunctionType.Relu,
                        )
                        if k < 2:
                            pst = psp.tile([128, 128], f32, tag="tps")
                            tp = nc.tensor.transpose(
                                out=pst[0:64, :],
                                in_=dst,
                                identity=ident[:],
                            )
                            add_dep_helper(tp.ins, e3.ins, sync=True, reason="agg ready")
                            dst_tile = xT0 if k == 0 else xT1
                            ro = 64 if k == 0 else 0
                            nc.scalar.copy(
                                out=dst_tile[ro : ro + 64, c * 128 : (c + 1) * 128],
                                in_=pst[0:64, :],
                            )

            if KDBG:
                nchan = 64 * (NBRUN + 1)
                nc.sync.dma_start(
                    out=DbgXc[:, 0 : JBLK * nchan].rearrange(
                        "p (j c) -> p j c", c=nchan
                    ),
                    in_=Xcat[:, :, 0:nchan],
                )
            # 4) final quad max-pool (into As, free now)
            if NBRUN == 3:
                red = nc.vector.tensor_reduce(
                    out=As[:].rearrange("p j c -> p (j c)"),
                    in_=Xcat[:].rearrange("p j (g f) -> p (j g) f", f=4),
                    op=mybir.AluOpType.max,
                    axis=mybir.AxisListType.X,
                )
                ow = nc.sync.dma_start(
                    out=OutP[:].rearrange("(j p) c -> p j c", p=128),
                    in_=As[:],
                )
                add_dep_helper(ow.ins, red.ins, sync=True, reason="out ready")
            else:
                nc.sync.dma_start(
                    out=OutP[:].rearrange("(j p) c -> p j c", p=128),
                    in_=As[:],
                )

    nc.compile()

    # ---- host-side shard + run ----
    in_maps = []
    for m in range(N_CORES):
        perm = perms[m]
        xp = np.zeros((SLOTS, 64), np.float32)
        sel = perm >= 0
        xp[sel] = x[perm[sel]]
        xnm = np.zeros((128, JBLK, 64), np.float32)
        s = np.arange(SLOTS)
        xnm[s % 128, s // 128, :] = xp
        in_map = {
            "XT0in": np.ascontiguousarray(xp.T),
            "Xnm": xnm.reshape(128, -1),
            "IdxP": idx16[m],
        }
        for k in range(3):
            C = CS[k]
            Wt_, Wb_ = Ws[k][:C], Ws[k][C:]
            comb = np.concatenate([Wt_ - Wb_, Wb_], axis=1)  # [C, 128]
            brow = np.concatenate([bs[k], np.zeros(64, np.float32)])[None, :]
            in_map[f"W{k}"] = np.ascontiguousarray(
                np.vstack([comb, brow]).astype(np.float32)
            )
        in_maps.append(in_map)

    if os.environ.get("BASS_SIM", "0") == "1":
        from concourse.bass_interp import MultiCoreSim

        sim = MultiCoreSim(nc, num_cores=N_CORES, num_workers=8)
        for m, core_sim in sim.cores.items():
            for name, val in in_maps[m].items():
                core_sim.tensor(name)[:] = val
        sim.simulate()
        out = np.zeros((N_NODES, 64), np.float32)
        for m in range(N_CORES):
            o = np.asarray(sim.cores[m].tensor("Out"))
            perm = perms[m]
            sel = perm >= 0
            out[perm[sel]] = o[sel]
        kernel._last_res = None
        return out

    trace = os.environ.get("BASS_KERNEL_TRACE", "0") == "1" and _setup_trace_hook()
    res = run_bass_kernel_spmd(
        nc, in_maps, core_ids=list(range(N_CORES)), trace=trace
    )
    out = np.zeros((N_NODES, 64), np.float32)
    for m in range(N_CORES):
        o = res.results[m]["Out"]
        perm = perms[m]
        sel = perm >= 0
        out[perm[sel]] = o[sel]
    kernel._last_res = res
    return out


# revision 46
# speedup vs baseline: 1.1015x; 1.0918x over previous
"""DenseGCN (3x EdgeConv max-aggr) on 8 TRN2 NeuronCores.

Key algebra: EdgeConv message relu([x_d, x_s - x_d] @ W + b) with W=[Wt;Wb]
= relu(x_d@(Wt-Wb) + x_s@Wb + b). relu and the per-dst add are monotone, so
segment_max commutes: agg[v] = relu(A[v] + b + max_{e:dst=v} B[src_e]) with
A = x@(Wt-Wb) + b, B = x@Wb -- node-level matmuls only, no per-edge matmul.

Sharding: dst-partition nodes (2500/core, padded to 2560 slots, sorted by
in-degree desc). Per block: one merged [C,128] matmul per node tile yields
[A|B] in PSUM, AllGather of the bf16 B table, then a handful of giant
multi-column indirect-DMA gathers into a bf16 SBUF slab (amortizes the ~1us
fixed SWDGE cost that dominated the per-round version), DVE tree-folds for
the segment max, and a final quad max-pool.
"""
import sys, os, types

sys.path.insert(0, "/opt/trn_rl_repo")

import numpy as np


def _setup_trace_hook():
    """Register the NTFF profile hook (missing antenv.axon_hooks shim) so
    run_bass_kernel_spmd(trace=True) works. Safe no-op on failure."""
    try:
        import antenv

        if "antenv.axon_hooks" not in sys.modules:
            m = types.ModuleType("antenv.axon_hooks")
            hook = [None]
            m.set_axon_ntff_profile_hook = lambda h: hook.__setitem__(0, h)
            m.get_axon_ntff_profile_hook = lambda: hook[0]
            sys.modules["antenv.axon_hooks"] = m
            antenv.axon_hooks = m
            from trn_agent_boot.trn_boot import _ntff_profile_via_ctypes

            m.set_axon_ntff_profile_hook(
                _ntff_profile_via_ctypes("/opt/axon/libaxon_pjrt.so")
            )
        import concourse.bass_utils as bu

        bu.upload_artifacts = lambda tmpdir: tmpdir
        return True
    except Exception:
        return False

N_NODES = 20000
N_EDGES = 320000
C_IN = 64
GROWTH = 64
N_BLOCKS = 3
N_CORES = 8
NL = N_NODES // N_CORES          # 2500 local nodes
SLOTS = 2560                     # padded to 20*128
JBLK = SLOTS // 128              # 20 slot-blocks
TBL_STRIDE = SLOTS               # per-core chunk in the AllGathered table
TBL_ROWS = N_CORES * TBL_STRIDE + 64  # + slack: the AG tail clobbers past-end
NGRP = 4                         # gather calls per block (pipeline granularity)


def _build_grid(edge_index):
    """Host-side index manipulation: degree-sort nodes per core, build the
    per-slot-block column layout (R_c columns for block c) and the int32
    gather index grid [8, 128, K], identical structure across cores."""
    src, dst = edge_index[0].astype(np.int64), edge_index[1].astype(np.int64)
    core_of = dst // NL
    perms = []
    slot_of = np.full(N_NODES, -1, np.int64)
    for m in range(N_CORES):
        lo, hi = m * NL, (m + 1) * NL
        deg = np.bincount(dst[(dst >= lo) & (dst < hi)] - lo, minlength=NL)
        order = np.argsort(-deg, kind="stable")
        nodes = order + lo
        perm = np.concatenate([nodes, np.full(SLOTS - NL, -1, np.int64)])
        perms.append(perm)
        slot_of[nodes] = np.arange(NL)
    grow = (np.arange(N_NODES) // NL) * TBL_STRIDE + slot_of  # node -> table row

    per_core = []
    degs = np.zeros((N_CORES, SLOTS), np.int64)
    for m in range(N_CORES):
        lo = m * NL
        mask = core_of == m
        s_rows = grow[src[mask]]
        sl = slot_of[dst[mask]]
        o = np.argsort(sl, kind="stable")
        sl, s_rows = sl[o], s_rows[o]
        starts = np.searchsorted(sl, np.arange(SLOTS))
        ends = np.searchsorted(sl, np.arange(SLOTS) + 1)
        degs[m] = ends - starts
        per_core.append((sl, s_rows, starts))

    # columns per slot-block: R_c = max degree within block across cores (>=1)
    Rs = [max(1, int(degs[:, c * 128 : (c + 1) * 128].max())) for c in range(JBLK)]
    col_base = np.concatenate([[0], np.cumsum(Rs)]).astype(np.int64)
    K = int(col_base[-1])

    # Pad a slot's unused rounds by REPEATING its first source row (max is
    # idempotent) -- repeated gathers of one shared dummy row misbehave in the
    # HW SWDGE ucode. Zero-degree slots (only the discarded pad slots here)
    # point at row 0; their outputs never leave the device unshuffled range.
    idx_all = np.full((N_CORES, 128, K), -1, np.int32)
    for m in range(N_CORES):
        sl, s_rows, starts = per_core[m]
        r = np.arange(len(sl)) - starts[sl]          # round of each edge
        cols = col_base[sl // 128] + r
        idx_all[m, sl % 128, cols] = s_rows
        first_src = np.zeros(SLOTS, np.int64)
        has = degs[m] > 0
        first_src[has] = s_rows[starts[np.where(has)[0]]]
        fs_pj = np.zeros((128, K), np.int64)
        for c in range(JBLK):
            fs_pj[:, col_base[c] : col_base[c + 1]] = first_src[
                c * 128 : (c + 1) * 128
            ][:, None]
        grid = idx_all[m]
        idx_all[m] = np.where(grid < 0, fs_pj, grid).astype(np.int32)

    # dma_gather int16 index layout, per <=1024-idx call (HW per-call limit):
    # call j covers slab cols [8j, 8j+8); its flat idx i = q*128 + p lives at
    # idx16[i % 16, 64j + i // 16], replicated across the 8 Q7 stripes.
    K_pad = ((K + 7) // 8) * 8
    ncalls = K_pad // 8
    idx16 = np.zeros((N_CORES, 128, K_pad * 8), np.int16)
    for m in range(N_CORES):
        flat = np.zeros(K_pad * 128, np.int64)
        flat[: K * 128] = idx_all[m].T.reshape(-1)    # (q*128 + p) order
        for j in range(ncalls):
            seg = flat[j * 1024 : (j + 1) * 1024]
            wrap = seg.reshape(-1, 16).T.astype(np.int16)   # [16, 64]
            idx16[m, :, j * 64 : (j + 1) * 64] = np.tile(wrap, (8, 1))
    return perms, Rs, col_base, K, K_pad, idx_all, idx16


def kernel(x, W0, b0, W1, b1, W2, b2, edge_index):
    import concourse.bacc as bacc
    import concourse.bass as bass
    import concourse.mybir as mybir
    import concourse.tile as tile
    from concourse.tile import add_dep_helper
    from concourse.masks import make_identity
    from concourse.bass_utils import run_bass_kernel_spmd
    from concourse.library_config import mlp as mlp_lib

    x = np.asarray(x, np.float32)
    Ws = [np.asarray(W0, np.float32), np.asarray(W1, np.float32), np.asarray(W2, np.float32)]
    bs = [np.asarray(b0, np.float32), np.asarray(b1, np.float32), np.asarray(b2, np.float32)]
    edge_index = np.asarray(edge_index)

    perms, Rs, col_base, K, K_pad, idx_all, idx16 = _build_grid(edge_index)
    NCALLS = K_pad // 8
    CS = [C_IN + k * GROWTH for k in range(N_BLOCKS)]   # 64,128,192

    # runs of adjacent slot-blocks with equal R (for batched tree folds)
    runs = []  # (c0, ncs, R)
    c = 0
    while c < JBLK:
        c2 = c
        while c2 + 1 < JBLK and Rs[c2 + 1] == Rs[c]:
            c2 += 1
        runs.append((c, c2 - c + 1, Rs[c]))
        c = c2 + 1

    f32 = mybir.dt.float32
    nc = bacc.Bacc(
        "TRN2", target_bir_lowering=False, debug=False, num_devices=N_CORES,
        dynamic_dma_scratch_size=32768, num_swdge_queues=4,
    )

    XT0in = nc.declare_dram_parameter("XT0in", [64, SLOTS], f32, isOutput=False)
    Xnm = nc.declare_dram_parameter("Xnm", [128, JBLK * 64], f32, isOutput=False)
    # Wc_k: rows 0:C = [Wt-Wb | Wb] ([C,128]), row C = [b | 0]
    Wp = [nc.declare_dram_parameter(f"W{k}", [CS[k] + 1, 128], f32, isOutput=False) for k in range(3)]
    IdxP = nc.declare_dram_parameter("IdxP", [128, K_pad * 8], mybir.dt.int16, isOutput=False)
    OutP = nc.declare_dram_parameter("Out", [SLOTS, 64], f32, isOutput=True)

    Btabs = [
        nc.dram_tensor(f"btab{i}", [TBL_ROWS, 64], f32, addr_space="Shared")
        for i in range(2)
    ]
    Bloc = nc.dram_tensor("bloc", [TBL_STRIDE, 64], f32)

    KDBG = os.environ.get("KDBG", "0") == "1"
    NBRUN = int(os.environ.get("KBLOCKS", "3"))
    if KDBG:
        DbgSlab = nc.declare_dram_parameter("DbgSlab", [128, K * 64], f32, isOutput=True)
        DbgAs = nc.declare_dram_parameter("DbgAs", [128, JBLK * 64], f32, isOutput=True)
        DbgXc = nc.declare_dram_parameter("DbgXc", [128, JBLK * 256], f32, isOutput=True)

    # W chunk map per block: (xt_tile_idx, xt_row0, kk, wslot)
    chunk_map = {
        0: [(0, 0, 64, 0)],
        1: [(0, 0, 128, 1)],
        2: [(0, 0, 128, 2), (1, 0, 64, 3)],
    }
    NWSLOT = 4

    with tile.TileContext(nc) as tc:
        with (
            tc.tile_pool(name="big", bufs=1) as bigp,
            tc.tile_pool(name="ps", bufs=4, space="PSUM") as psp,
        ):
            xT0 = bigp.tile([128, SLOTS], f32, tag="xT0")
            xT1 = bigp.tile([128, SLOTS], f32, tag="xT1")
            Xcat = bigp.tile([128, JBLK, 256], f32, tag="Xcat")
            As = bigp.tile([128, JBLK, 64], f32, tag="As")
            Bs = bigp.tile([128, JBLK, 64], f32, tag="Bs")
            slab = bigp.tile([128, K_pad, 64], f32, tag="slab")
            Idx = bigp.tile([128, K_pad * 8], mybir.dt.int16, tag="Idx")
            Wt = bigp.tile([128, NWSLOT * 128], f32, tag="Wt")
            ident = bigp.tile([128, 128], f32, tag="ident")

            ll = nc.gpsimd.load_library(mlp_lib)  # dma_gather ucode
            make_identity(nc, ident[:])
            nc.sync.dma_start(out=xT0[0:64, :], in_=XT0in[:])
            nc.sync.dma_start(
                out=Xcat[:, :, 0:64],
                in_=Xnm[:].rearrange("p (j c) -> p j c", c=64),
            )
            nc.sync.dma_start(out=Idx[:], in_=IdxP[:])
            for k in range(3):
                C = CS[k]
                for (xti, row0, kk, wslot) in chunk_map[k]:
                    wr0 = 0 if wslot != 3 else 128
                    nc.sync.dma_start(
                        out=Wt[0:kk, wslot * 128 : (wslot + 1) * 128],
                        in_=Wp[k][wr0 : wr0 + kk, :],
                    )
            war_dep = {0: [], 1: []}
            gidx = 0  # global SWDGE gather counter: keeps Tile's DMASW lane
            #           rotation (mod 8) aligned with our queue rotation (mod 4)
            for k in range(NBRUN):
                Btab = Btabs[k % 2]
                chunks = chunk_map[k]

                # 1) merged [A|B] matmuls; B half -> Bloc asap (gates the AG)
                bws = []
                for t in range(JBLK):
                    ps = psp.tile([128, 128], f32, tag="mm")
                    for ci, (xti, row0, kk, wslot) in enumerate(chunks):
                        xt_tile = xT0 if xti == 0 else xT1
                        nc.tensor.matmul(
                            out=ps[:, :],
                            lhsT=xt_tile[row0 : row0 + kk, t * 128 : (t + 1) * 128],
                            rhs=Wt[0:kk, wslot * 128 : (wslot + 1) * 128],
                            start=(ci == 0),
                            stop=(ci == len(chunks) - 1),
                        )
                    cpB = nc.scalar.copy(out=Bs[:, t, :], in_=ps[:, 64:128])
                    bw = nc.sync.dma_start(
                        out=Bloc[t * 128 : (t + 1) * 128, :], in_=Bs[:, t, :]
                    )
                    bws.append(bw)
                    # A columns already hold x@(Wt-Wb); just move PSUM -> SBUF
                    nc.vector.tensor_copy(out=As[:, t, :], in_=ps[:, 0:64])

                ag = nc.gpsimd.collective_compute(
                    "AllGather", mybir.AluOpType.bypass,
                    replica_groups=[list(range(N_CORES))],
                    ins=[Bloc[:]],
                    outs=[Btab[0 : N_CORES * TBL_STRIDE, :]],
                )
                for bw in bws:
                    add_dep_helper(ag.ins, bw.ins, sync=True, reason="AG after writes")
                for d in war_dep[k % 2]:
                    add_dep_helper(ag.ins, d.ins, sync=True, reason="WAR: prior reads done")

                # 2) SWDGE row-gathers (<=1024 idx/call, 4 queues round-robin)
                gcalls = []
                for j in range(NCALLS):
                    gcall = nc.gpsimd.dma_gather(
                        slab[:, j * 8 : (j + 1) * 8, :],
                        Btab[0 : N_CORES * TBL_STRIDE, :],
                        Idx[:, j * 64 : (j + 1) * 64],
                        1024,
                        1024,
                        64,
                        queue_num=gidx % 4,
                    )
                    gidx += 1
                    add_dep_helper(gcall.ins, ag.ins, sync=True, reason="table ready")
                    if k == 0 and j == 0:
                        add_dep_helper(gcall.ins, ll.ins, sync=True, reason="ucode")
                    gcalls.append(gcall)
                war_dep[k % 2] = gcalls
                if KDBG and k == NBRUN - 1:
                    nc.sync.dma_start(
                        out=DbgSlab[:], in_=slab[:, 0:K, :].rearrange("p q f -> p (q f)")
                    )
                    nc.sync.dma_start(
                        out=DbgAs[:], in_=As[:].rearrange("p j c -> p (j c)")
                    )

                # 3) per run: tree-fold the segments, then epilogue+transpose;
                #    tile per-AP deps pipeline these behind the right calls
                for (c0, ncs, R) in runs:
                    q0 = int(col_base[c0])
                    seg = slab[:, q0 : q0 + ncs * R, :].rearrange(
                        "p (c r) f -> p c r f", r=R
                    )
                    L = R
                    while L > 1:
                        h = L // 2
                        nc.vector.tensor_tensor(
                            out=seg[:, :, 0:h, :],
                            in0=seg[:, :, 0:h, :],
                            in1=seg[:, :, L - h : L, :],
                            op=mybir.AluOpType.max,
                        )
                        L -= h
                    for c in range(c0, c0 + ncs):
                        cb = int(col_base[c])
                        dst = Xcat[:, c, 64 * (k + 1) : 64 * (k + 2)]
                        nc.vector.tensor_tensor(
                            out=dst, in0=As[:, c, :], in1=slab[:, cb, :],
                            op=mybir.AluOpType.add,
                        )
                        e3 = nc.scalar.activation(
                            out=dst, in_=dst,
                            func=mybir.ActivationFunctionType.Relu,
                        )
                        if k < 2:
                            pst = psp.tile([128, 128], f32, tag="tps")
                            tp = nc.tensor.transpose(
                                out=pst[0:64, :],
                                in_=dst,
                                identity=ident[:],
                            )
                            add_dep_helper(tp.ins, e3.ins, sync=True, reason="agg ready")
                            dst_tile = xT0 if k == 0 else xT1
                            ro = 64 if k == 0 else 0
                            nc.scalar.copy(
                                out=dst_tile[ro : ro + 64, c * 128 : (c + 1) * 128],
                                in_=pst[0:64, :],
                            )

            if KDBG:
                nchan = 64 * (NBRUN + 1)
                nc.sync.dma_start(
                    out=DbgXc[:, 0 : JBLK * nchan].rearrange(
                        "p (j c) -> p j c", c=nchan
                    ),
                    in_=Xcat[:, :, 0:nchan],
                )
            # 4) final quad max-pool (into As, free now)
            if NBRUN == 3:
                red = nc.vector.tensor_reduce(
                    out=As[:].rearrange("p j c -> p (j c)"),
                    in_=Xcat[:].rearrange("p j (g f) -> p (j g) f", f=4),
                    op=mybir.AluOpType.max,
                    axis=mybir.AxisListType.X,
                )
                ow = nc.sync.dma_start(
                    out=OutP[:].rearrange("(j p) c -> p j c", p=128),
                    in_=As[:],
                )
                add_dep_helper(ow.ins, red.ins, sync=True, reason="out ready")
            else:
                nc.sync.dma_start(
                    out=OutP[:].rearrange("(j p) c -> p j c", p=128),
                    in_=As[:],
                )

    nc.compile()

    # ---- host-side shard + run ----
    in_maps = []
    for m in range(N_CORES):
        perm = perms[m]
        xp = np.zeros((SLOTS, 64), np.float32)
        sel = perm >= 0
        xp[sel] = x[perm[sel]]
        xnm = np.zeros((128, JBLK, 64), np.float32)
        s = np.arange(SLOTS)
        xnm[s % 128, s // 128, :] = xp
        in_map = {
            "XT0in": np.ascontiguousarray(xp.T),
            "Xnm": xnm.reshape(128, -1),
            "IdxP": idx16[m],
        }
        for k in range(3):
            C = CS[k]
            Wt_, Wb_ = Ws[k][:C], Ws[k][C:]
            comb = np.concatenate([Wt_ - Wb_, Wb_], axis=1)  # [C, 128]
            brow = np.concatenate([bs[k], np.zeros(64, np.float32)])[None, :]
            in_map[f"W{k}"] = np.ascontiguousarray(
                np.vstack([comb, brow]).astype(np.float32)
            )
        in_maps.append(in_map)

    if os.environ.get("BASS_SIM", "0") == "1":
        from concourse.bass_interp import MultiCoreSim

        sim = MultiCoreSim(nc, num_cores=N_CORES, num_workers=8)
        for m, core_sim in sim.cores.items():
            for name, val in in_maps[m].items():
                core_sim.tensor(name)[:] = val
        sim.simulate()
        out = np.zeros((N_NODES, 64), np.float32)
        for m in range(N_CORES):
            o = np.asarray(sim.cores[m].tensor("Out"))
            perm = perms[m]
            sel = perm >= 0
            out[perm[sel]] = o[sel]
        kernel._last_res = None
        return out

    trace = os.environ.get("BASS_KERNEL_TRACE", "0") == "1" and _setup_trace_hook()
    res = run_bass_kernel_spmd(
        nc, in_maps, core_ids=list(range(N_CORES)), trace=trace
    )
    out = np.zeros((N_NODES, 64), np.float32)
    for m in range(N_CORES):
        o = res.results[m]["Out"]
        perm = perms[m]
        sel = perm >= 0
        out[perm[sel]] = o[sel]
    kernel._last_res = res
    return out
